# revision 13
# baseline (speedup 1.0000x reference)
# Distributed Trainium2 attention-layer kernel (8 NeuronCores).
#
# Sharding: core c in 0..7 handles (batch b = c//4, head group g = c%4).
# Each core computes q/k/v projections for its 4 heads (columns 256g:256g+256
# of Wq/Wk/Wv), rotary, scores^T, softmax (denominator via a ones-column in
# V), probs@V, and a partial out = attn_local @ Wo[rows of g]. The host sums
# the 4 group partials per batch (the tensor-parallel all-reduce, done on the
# host since full I/O passes through it anyway).
#
# v4 schedule: 512-query windows (8 windows x 16 key-chunks = 128 steps),
# paced by a 128-instruction ScalarE exp stream (FD=1024 per step: both
# heads of the pair side by side in one 2-bank PSUM tile).
#  - PSUM plan (8 banks): sc ring-2 (2x2 banks), outT (2x1), pp (1), op (1).
#  - scores: the two 64-row MMs of a step sit in different PE row groups
#    (base partitions 0/64) and different PSUM banks, emitted adjacently so
#    they co-stream (observed dstart ~3ns); ring-2 means both unblock the
#    moment the exp two steps back retires, so the exp stream only ever
#    waits on scores.
#  - projections/out-proj are emitted at fixed steps chosen so every read
#    is emitted after its producer (correctness is emission-order-robust,
#    not planner-timing-dependent).
#  - epilogue: outT evacuation right after the window; the normalize chain
#    (recip + broadcast + mul) is deferred to the next window's kc=1 and
#    the 1/den broadcast runs as two K=1 PE matmuls instead of the slow
#    serialized gpsimd partition_broadcast chain.
#  - head: full-array warmup MMs (a [1,x] stationary does not register as
#    PE activity, so the HAM clock gate never releases to 2.4 GHz).
#
# Self-contained: shapes hardcoded, no sibling imports.

import functools

import numpy as np
import ml_dtypes

import concourse.bass as bass
import concourse.bacc as bacc
import concourse.tile as tile
import concourse.mybir as mybir
from concourse.bass_utils import run_bass_kernel_spmd

BF16 = mybir.dt.bfloat16
F32 = mybir.dt.float32

H = 16
D = 64
HID = 1024
ROT = 32
B = 2
S = 2048
NCORES = 8
HPC = 4          # heads per core
LCOL = HPC * D   # 256 local columns

QW = 512         # query-window width
NW = S // QW     # 4 qb windows per head pair
PV_LAG = 3       # PV drains this many steps behind the exp stream
N_STEPS = 2 * NW * 16   # 128

LAST_RESULT = None  # BassKernelResults of the most recent run (for test.py)


@functools.lru_cache(maxsize=4)
def _build(use_qkb: bool, use_vb: bool, use_ab: bool):
    nc = bacc.Bacc("TRN2", target_bir_lowering=False, debug=False)

    xT = nc.dram_tensor("xT", [HID, S], BF16, kind="ExternalInput")
    wq = nc.dram_tensor("wq", [128, 8 * LCOL], BF16, kind="ExternalInput")
    wk = nc.dram_tensor("wk", [128, 8 * LCOL], BF16, kind="ExternalInput")
    wv = nc.dram_tensor("wv", [128, 8 * LCOL], BF16, kind="ExternalInput")
    wo = nc.dram_tensor("wo", [128, 2 * HID], BF16, kind="ExternalInput")
    rotm = nc.dram_tensor("rotm", [128, S], BF16, kind="ExternalInput")
    if use_qkb:
        bqd = nc.dram_tensor("bqd", [128, 2], F32, kind="ExternalInput")
        bkd = nc.dram_tensor("bkd", [128, 2], F32, kind="ExternalInput")
    if use_vb:
        bvd = nc.dram_tensor("bvd", [128, LCOL], F32, kind="ExternalInput")
    if use_ab:
        expb = nc.dram_tensor("expb", [S, S], F32, kind="ExternalInput")
    out = nc.dram_tensor("out", [S, HID], BF16, kind="ExternalOutput")

    with tile.TileContext(nc) as tc:
        with (
            tc.tile_pool(name="per", bufs=1) as per,
            tc.tile_pool(name="ex", bufs=10) as exp_pool,
            tc.tile_pool(name="asc", bufs=4) as asc_pool,
            tc.tile_pool(name="ps", bufs=2, space="PSUM") as ps,
        ):
            # ---- persistent SBUF residents ----
            xT_sb = per.tile([128, 8 * S], BF16)        # hid-chunk h at cols h*S
            wq_sb = per.tile([128, 8 * LCOL], BF16)     # hid-chunk h at cols h*LCOL
            wk_sb = per.tile([128, 8 * LCOL], BF16)
            wv_sb = per.tile([128, 8 * LCOL], BF16)
            wo_sb = per.tile([128, 2 * HID], BF16)      # col-chunk c at cols c*HID
            rotm_sb = per.tile([128, S], BF16)
            qT_sb = per.tile([128, 2 * S], BF16)        # col-chunk c at cols c*S
            kT_sb = per.tile([128, 2 * S], BF16)
            v_sb = per.tile([128, 16 * (HPC * 65)], BF16)  # k-chunk kc: 4 heads x 65
            attnT_sb = per.tile([128, 2 * S], BF16)     # raw (unnormalized) attn^T
            asc_sb = per.tile([128, 2 * S], BF16)       # normalized attn^T
            ones_sb = per.tile([128, 512], BF16)        # warmup operands
            onesf_sb = per.tile([1, 128], F32)          # fp32 broadcast ones

            # warmup first: full-array matmuls so the PE HAM clock gate sees
            # real activity and releases to 2.4 GHz before the projections.
            nc.vector.memset(ones_sb[:], 1.0)
            nc.vector.memset(onesf_sb[:], 1.0)
            wu = ps.tile([128, 512], F32, tag="op", bufs=1, name="wu")
            for i in range(8):
                nc.tensor.matmul(wu[:, 0:448], ones_sb[:, 0:128],
                                 ones_sb[:, 0:448],
                                 start=(i == 0), stop=(i == 7))

            # ---- input DMAs on two queues for earliest first-window start
            xT_r = xT.rearrange("(c p) n -> p c n", p=128)
            xTs_r = xT_sb[:].rearrange("p (c n) -> p c n", c=8)
            nc.gpsimd.dma_start(wq_sb[:], wq[:])
            nc.sync.dma_start(xTs_r[:, :, 0:512], xT_r[:, :, 0:512])
            nc.gpsimd.dma_start(rotm_sb[:], rotm[:])
            nc.gpsimd.dma_start(wk_sb[:], wk[:])
            nc.gpsimd.dma_start(wv_sb[:], wv[:])
            nc.sync.dma_start(xTs_r[:, :, 512:1024], xT_r[:, :, 512:1024])
            nc.sync.dma_start(xTs_r[:, :, 1024:2048], xT_r[:, :, 1024:2048])
            nc.sync.dma_start(wo_sb[:], wo[:])
            if use_qkb:
                bq_sb = per.tile([128, 2], F32)
                bk_sb = per.tile([128, 2], F32)
                nc.sync.dma_start(bq_sb[:], bqd[:])
                nc.sync.dma_start(bk_sb[:], bkd[:])
            if use_vb:
                bv_sb = per.tile([128, LCOL], F32)
                nc.sync.dma_start(bv_sb[:], bvd[:])

            # ones columns of v (65th col of each head block)
            v_blocks = v_sb[:].rearrange("p (j c) -> p j c", c=65)
            nc.vector.memset(v_blocks[:, :, 64:65], 1.0)

            # ---- projection pieces ----
            def proj_qk(which, c, s4):
                """q/k projection for 512-wide s-block s4 of col-chunk c:
                8 accumulating matmuls, then rotary multiply into qT/kT."""
                w_sb, dst = (wq_sb, qT_sb) if which == "q" else (wk_sb, kT_sb)
                s0 = s4 * 512
                pp = ps.tile([128, 512], F32, tag="pp", bufs=1,
                             name=f"pp_{which}{c}{s4}")
                for h in range(8):
                    nc.tensor.matmul(
                        pp[:],
                        w_sb[:, h * LCOL + c * 128:h * LCOL + (c + 1) * 128],
                        xT_sb[:, h * S + s0:h * S + s0 + 512],
                        start=(h == 0),
                        stop=(h == 7),
                    )
                if use_qkb:
                    bias_ap = (bq_sb if which == "q" else bk_sb)[:, c:c + 1]
                    nc.scalar.add(pp[:], pp[:], bias_ap)
                nc.vector.tensor_mul(
                    dst[:, c * S + s0:c * S + s0 + 512],
                    pp[:],
                    rotm_sb[:, s0:s0 + 512],
                )

            def proj_v(j):
                """v projection for s-chunk j (128 rows): 8 N=256 matmuls."""
                vp = ps.tile([128, LCOL], F32, tag="pp", bufs=1,
                             name=f"vp_{j}")
                for h in range(8):
                    nc.tensor.matmul(
                        vp[:],
                        xT_sb[:, h * S + j * 128:h * S + (j + 1) * 128],
                        wv_sb[:, h * LCOL:(h + 1) * LCOL],
                        start=(h == 0),
                        stop=(h == 7),
                    )
                dst = v_sb[:, j * (HPC * 65):(j + 1) * (HPC * 65)].rearrange(
                    "p (h c) -> p h c", c=65
                )[:, :, 0:64]
                src = vp[:].rearrange("p (h c) -> p h c", c=64)
                if use_vb:
                    nc.vector.tensor_add(
                        dst, src, bv_sb[:].rearrange("p (h c) -> p h c", c=64)
                    )
                else:
                    nc.vector.tensor_copy(dst, src)

            def out_proj_half(j, n, last_grp):
                """out[j*128:(j+1)*128, n*512:(n+1)*512] = sum_c asc@wo."""
                # alternate PSUM banks so consecutive halves double-buffer
                op = ps.tile([128, 512], F32,
                             tag=("op" if (2 * j + n) % 2 == 0 else "pp"),
                             bufs=1, name=f"op{j}{n}")
                for c in range(2):
                    nc.tensor.matmul(
                        op[:],
                        asc_sb[:, c * S + j * 128:c * S + (j + 1) * 128],
                        wo_sb[:, c * HID + n * 512:c * HID + (n + 1) * 512],
                        start=(c == 0),
                        stop=(c == 1),
                    )
                ost = asc_pool.tile([128, 512], BF16, tag="ost", bufs=4,
                                    name=f"ost{j}{n}")
                if last_grp and (2 * j + n) % 2 == 0:
                    # ScalarE is idle after the final exp; split the tail
                    # evacuations across ScalarE and VectorE
                    nc.scalar.copy(ost[:], op[:])
                else:
                    nc.vector.tensor_copy(ost[:], op[:])
                dma_eng = nc.sync if (j + n) % 2 == 0 else nc.gpsimd
                dma_eng.dma_start(
                    out[j * 128:(j + 1) * 128, n * 512:(n + 1) * 512],
                    ost[:],
                )

            # ---- fixed-step fill schedule (emission order == dep order) ----
            fills = [[] for _ in range(N_STEPS)]

            def at(t, fn):
                fills[max(0, min(t, N_STEPS - 1))].append(fn)

            # v(j) emitted at step j; consumed by the PV drain at step j+LAG.
            for j in range(16):
                at(j, lambda j=j: proj_v(j))
            # k(0,s4) emitted 4 steps before its first consuming scores.
            at(0, lambda: proj_qk("k", 0, 1))
            at(4, lambda: proj_qk("k", 0, 2))
            at(8, lambda: proj_qk("k", 0, 3))
            # q(0,qb) emitted a few steps before window (0,qb).
            at(12, lambda: proj_qk("q", 0, 1))
            at(27, lambda: proj_qk("q", 0, 2))
            at(43, lambda: proj_qk("q", 0, 3))
            # k(1,*) and q(1,0) during windows 2-3.
            at(34, lambda: proj_qk("k", 1, 0))
            at(38, lambda: proj_qk("k", 1, 1))
            at(42, lambda: proj_qk("k", 1, 2))
            at(46, lambda: proj_qk("k", 1, 3))
            at(50, lambda: proj_qk("q", 1, 0))
            at(74, lambda: proj_qk("q", 1, 1))
            at(90, lambda: proj_qk("q", 1, 2))
            at(106, lambda: proj_qk("q", 1, 3))
            # out-proj groups 0-2 into windows 5-7, strictly after group g's
            # c1 normalize (norm_b of window 4+g runs at step (5+g)*16+4).
            for g in range(3):
                base = (5 + g) * 16 + 5
                i = 0
                for j in range(4 * g, 4 * g + 4):
                    for n in range(2):
                        at(base + i, lambda j=j, n=n: out_proj_half(j, n, False))
                        i += 1

            # ---- head: first window's q/k pieces ----
            proj_qk("q", 0, 0)
            proj_qk("k", 0, 0)

            deferred_norm_a = None
            deferred_norm_b = None

            def make_norm(w, p, qb, w0, outT, last_win):
                state = {}

                def norm_a():
                    # 1/den, then broadcast to the two 64-row head substripes
                    # (gpsimd; latency hidden by the norm_b deferral).
                    dt = asc_pool.tile([1, 1024], F32, tag="dt", bufs=2,
                                       name=f"dt{w}")
                    if last_win:
                        nc.scalar.copy(dt[:, 0:512], outT[0][64:65, :])
                        nc.scalar.copy(dt[:, 512:1024], outT[1][64:65, :])
                    else:
                        nc.vector.tensor_copy(dt[:, 0:512], outT[0][64:65, :])
                        nc.vector.tensor_copy(dt[:, 512:1024], outT[1][64:65, :])
                    rt = asc_pool.tile([1, 1024], F32, tag="rt", bufs=2,
                                       name=f"rt{w}")
                    nc.vector.reciprocal_approx_fast(rt[:], dt[:])
                    if last_win:
                        # low-latency path: bf16 K=1 PE matmuls into PSUM
                        rtb = asc_pool.tile([1, 1024], BF16, tag="rtb", bufs=2,
                                            name=f"rtb{w}")
                        nc.vector.tensor_copy(rtb[:], rt[:])
                        rb = ps.tile([128, 512], F32, tag="op", bufs=1,
                                     name=f"rb{w}")
                        nc.tensor.matmul(rb[0:64, :], ones_sb[0:1, 0:64],
                                         rtb[0:1, 0:512],
                                         start=True, stop=True)
                        nc.tensor.matmul(rb[64:128, :], ones_sb[0:1, 64:128],
                                         rtb[0:1, 512:1024],
                                         start=True, stop=True,
                                         skip_group_check=True)
                    else:
                        rb = asc_pool.tile([128, 512], F32, tag="rb", bufs=2,
                                           name=f"rb{w}")
                        rbt = asc_pool.tile([64, 512], F32, tag="rbt", bufs=2,
                                            name=f"rbt{w}")
                        nc.gpsimd.partition_broadcast(rb[0:64, :],
                                                      rt[:, 0:512],
                                                      channels=64)
                        nc.gpsimd.partition_broadcast(rbt[:],
                                                      rt[:, 512:1024],
                                                      channels=64)
                        nc.gpsimd.dma_start(rb[64:128, :], rbt[:])
                    state["rb"] = rb

                def norm_b():
                    nc.vector.tensor_mul(asc_sb[:, w0:w0 + 512],
                                         attnT_sb[:, w0:w0 + 512],
                                         state["rb"][:])
                return norm_a, norm_b

            # ---- main loop: 8 windows x 16 key chunks ----
            for w in range(2 * NW):
                p, qb = divmod(w, NW)
                w0 = p * S + qb * QW
                outT = [
                    ps.tile([65, 512], F32, tag="outT", bufs=2,
                            name=f"outT{w}{hi}")
                    for hi in range(2)
                ]
                pend = []  # (exp_tile, kc) awaiting PV
                for kc in range(16):
                    t = w * 16 + kc
                    sc = ps.tile([128, 1024], F32, tag="sc", name=f"sc{w}{kc}")
                    for hi in range(2):
                        off = hi * 64
                        nc.tensor.matmul(
                            sc[:, hi * 512:(hi + 1) * 512],
                            kT_sb[off:off + 64,
                                  p * S + kc * 128:p * S + (kc + 1) * 128],
                            qT_sb[off:off + 64, w0:w0 + QW],
                            start=True,
                            stop=True,
                        )
                    ex = exp_pool.tile([128, 1024], BF16, tag="ex",
                                       name=f"ex{w}{kc}")
                    nc.scalar.activation(
                        ex[:], sc[:], mybir.ActivationFunctionType.Exp,
                        scale=0.125,
                    )
                    if use_ab:
                        ebt = exp_pool.tile([128, 512], F32, tag="ebt",
                                            bufs=2, name=f"ebt{w}{kc}")
                        nc.sync.dma_start(
                            ebt[:],
                            expb[kc * 128:(kc + 1) * 128,
                                 qb * QW:(qb + 1) * QW],
                        )
                        for hi in range(2):
                            nc.vector.tensor_mul(
                                ex[:, hi * 512:(hi + 1) * 512],
                                ex[:, hi * 512:(hi + 1) * 512],
                                ebt[:],
                            )
                    pend.append((ex, kc))
                    while len(pend) > PV_LAG:
                        exq, kcq = pend.pop(0)
                        for hi in range(2):
                            hq = 2 * p + hi
                            nc.tensor.matmul(
                                outT[hi][:],
                                v_sb[:, kcq * (HPC * 65) + hq * 65:
                                     kcq * (HPC * 65) + hq * 65 + 65],
                                exq[:, hi * 512:(hi + 1) * 512],
                                start=(kcq == 0),
                                stop=(kcq == 15),
                            )
                    if kc == 1 and deferred_norm_a is not None:
                        deferred_norm_a()
                        deferred_norm_a = None
                    if kc == 4 and deferred_norm_b is not None:
                        deferred_norm_b()
                        deferred_norm_b = None
                    for fn in fills[t]:
                        fn()
                for exq, kcq in pend:
                    for hi in range(2):
                        hq = 2 * p + hi
                        nc.tensor.matmul(
                            outT[hi][:],
                            v_sb[:, kcq * (HPC * 65) + hq * 65:
                                 kcq * (HPC * 65) + hq * 65 + 65],
                            exq[:, hi * 512:(hi + 1) * 512],
                            start=(kcq == 0),
                            stop=(kcq == 15),
                        )
                pend = []
                # ---- epilogue phase 1: evacuate outT (frees PSUM) ----
                last_win = w == 2 * NW - 1
                nc.vector.tensor_copy(attnT_sb[0:64, w0:w0 + 512],
                                      outT[0][0:64, :])
                atmp = asc_pool.tile([64, 512], BF16, tag="atmp", bufs=2,
                                     name=f"atmp{w}")
                nc.vector.tensor_copy(atmp[:], outT[1][0:64, :])
                nc.gpsimd.dma_start(attnT_sb[64:128, w0:w0 + 512], atmp[:])
                # phase 2 (den recip/broadcast/normalize) deferred into the
                # next window so it does not head-of-line-block VectorE.
                norm_a, norm_b = make_norm(w, p, qb, w0, outT, last_win)
                if last_win:
                    norm_a()
                    norm_b()
                else:
                    deferred_norm_a = norm_a
                    deferred_norm_b = norm_b

            # ---- tail: last out-proj group after the final normalize ----
            for j in range(12, 16):
                for n in range(2):
                    out_proj_half(j, n, True)

    nc.compile()
    return nc


def _prep_core(c, x, sinusoids, attention_bias, Wq, bq, Wk, bk, Wv, bv, Wo,
               use_qkb, use_vb, use_ab):
    b, g = divmod(c, HPC)
    cols = slice(g * LCOL, (g + 1) * LCOL)
    bf = ml_dtypes.bfloat16
    m = {}
    m["xT"] = np.ascontiguousarray(x[b].T).astype(bf)
    def shuf(w):  # [1024, n] -> [128, 8*n]: row p = concat_h w[h*128+p, :]
        n = w.shape[1]
        return np.ascontiguousarray(
            w.reshape(8, 128, n).transpose(1, 0, 2).reshape(128, 8 * n))
    m["wq"] = shuf(Wq[:, cols]).astype(bf)
    m["wk"] = shuf(Wk[:, cols]).astype(bf)
    m["wv"] = shuf(Wv[:, cols]).astype(bf)
    wo_l = Wo[cols, :]  # [256, 1024] -> [128, 2*1024]
    m["wo"] = np.ascontiguousarray(
        wo_l.reshape(2, 128, HID).transpose(1, 0, 2).reshape(128, 2 * HID)
    ).astype(bf)
    sign = np.where(np.arange(ROT) % 2 == 0, -1.0, 1.0).astype(np.float32)
    mult = sinusoids[b, 1] + sign[None, :] * sinusoids[b, 0]   # [S, ROT]
    rotm = np.ones((128, S), dtype=np.float32)
    rotm[0:ROT] = mult.T
    rotm[64:64 + ROT] = mult.T
    m["rotm"] = rotm.astype(bf)
    if use_qkb:
        m["bqd"] = np.ascontiguousarray(
            bq[cols].reshape(2, 128).T).astype(np.float32)
        m["bkd"] = np.ascontiguousarray(
            bk[cols].reshape(2, 128).T).astype(np.float32)
    if use_vb:
        m["bvd"] = np.broadcast_to(
            bv[cols].astype(np.float32), (128, LCOL)).copy()
    if use_ab:
        m["expb"] = np.ascontiguousarray(
            np.exp(attention_bias[b, 0].astype(np.float32)).T)
    return m


def kernel(x, sinusoids, attention_bias, Wq, bq, Wk, bk, Wv, bv, Wo):
    global LAST_RESULT
    x = np.asarray(x, dtype=np.float32)
    sinusoids = np.asarray(sinusoids, dtype=np.float32)
    attention_bias = np.asarray(attention_bias, dtype=np.float32)
    Wq, Wk, Wv, Wo = (np.asarray(w, dtype=np.float32) for w in (Wq, Wk, Wv, Wo))
    bq, bk, bv = (np.asarray(v, dtype=np.float32) for v in (bq, bk, bv))

    use_qkb = bool(np.any(bq) or np.any(bk))
    use_vb = bool(np.any(bv))
    use_ab = bool(np.any(attention_bias))

    nc = _build(use_qkb, use_vb, use_ab)
    in_maps = [
        _prep_core(c, x, sinusoids, attention_bias, Wq, bq, Wk, bk, Wv, bv, Wo,
                   use_qkb, use_vb, use_ab)
        for c in range(NCORES)
    ]
    import os as _os
    res = run_bass_kernel_spmd(
        nc, in_maps, core_ids=list(range(NCORES)),
        tmpdir=_os.environ.get("BASS_TMPDIR"),
    )
    LAST_RESULT = res
    outs = [r["out"].astype(np.float32) for r in res.results]
    full = np.empty((B, S, HID), dtype=np.float32)
    for b in range(B):
        full[b] = outs[4 * b] + outs[4 * b + 1] + outs[4 * b + 2] + outs[4 * b + 3]
    return full


# revision 23
# speedup vs baseline: 1.0043x; 1.0043x over previous
# Distributed Trainium2 attention-layer kernel (8 NeuronCores).
#
# Sharding: core c in 0..7 handles (batch b = c//4, head group g = c%4).
# Each core computes q/k/v projections for its 4 heads (columns 256g:256g+256
# of Wq/Wk/Wv), rotary, scores^T, softmax (denominator via a ones-column in
# V), probs@V, and a partial out = attn_local @ Wo[rows of g]. The host sums
# the 4 group partials per batch (the tensor-parallel all-reduce, done on the
# host since full I/O passes through it anyway).
#
# v4 schedule: 512-query windows (8 windows x 16 key-chunks = 128 steps),
# paced by a 128-instruction ScalarE exp stream (FD=1024 per step: both
# heads of the pair side by side in one 2-bank PSUM tile).
#  - PSUM plan (8 banks): sc ring-2 (2x2 banks), outT (2x1), pp (1), op (1).
#  - scores: the two 64-row MMs of a step sit in different PE row groups
#    (base partitions 0/64) and different PSUM banks, emitted adjacently so
#    they co-stream (observed dstart ~3ns); ring-2 means both unblock the
#    moment the exp two steps back retires, so the exp stream only ever
#    waits on scores.
#  - projections/out-proj are emitted at fixed steps chosen so every read
#    is emitted after its producer (correctness is emission-order-robust,
#    not planner-timing-dependent).
#  - epilogue: outT evacuation right after the window; the normalize chain
#    (recip + broadcast + mul) is deferred to the next window's kc=1 and
#    the 1/den broadcast runs as two K=1 PE matmuls instead of the slow
#    serialized gpsimd partition_broadcast chain.
#  - head: full-array warmup MMs (a [1,x] stationary does not register as
#    PE activity, so the HAM clock gate never releases to 2.4 GHz).
#
# Self-contained: shapes hardcoded, no sibling imports.

import functools

import numpy as np
import ml_dtypes

import concourse.bass as bass
import concourse.bacc as bacc
import concourse.tile as tile
import concourse.mybir as mybir
from concourse.bass_utils import run_bass_kernel_spmd

BF16 = mybir.dt.bfloat16
F32 = mybir.dt.float32

H = 16
D = 64
HID = 1024
ROT = 32
B = 2
S = 2048
NCORES = 8
HPC = 4          # heads per core
LCOL = HPC * D   # 256 local columns

QW = 512         # query-window width
NW = S // QW     # 4 qb windows per head pair
PV_LAG = 3       # PV drains this many steps behind the exp stream
N_STEPS = 2 * NW * 16   # 128

LAST_RESULT = None  # BassKernelResults of the most recent run (for test.py)


@functools.lru_cache(maxsize=4)
def _build(use_qkb: bool, use_vb: bool, use_ab: bool):
    nc = bacc.Bacc("TRN2", target_bir_lowering=False, debug=False)

    # x^T in three host-packed region tensors (contiguous DMA rows: 8/8/16KB)
    xh = nc.dram_tensor("xh", [128, 8 * 512], BF16, kind="ExternalInput")
    xm = nc.dram_tensor("xm", [128, 8 * 512], BF16, kind="ExternalInput")
    xt2 = nc.dram_tensor("xt2", [128, 8 * 1024], BF16, kind="ExternalInput")
    wq = nc.dram_tensor("wq", [128, 8 * LCOL], BF16, kind="ExternalInput")
    wk = nc.dram_tensor("wk", [128, 8 * LCOL], BF16, kind="ExternalInput")
    wv = nc.dram_tensor("wv", [128, 8 * LCOL], BF16, kind="ExternalInput")
    wo = nc.dram_tensor("wo", [128, 2 * HID], BF16, kind="ExternalInput")
    rotm = nc.dram_tensor("rotm", [128, S], BF16, kind="ExternalInput")
    if use_qkb:
        bqd = nc.dram_tensor("bqd", [128, 2], F32, kind="ExternalInput")
        bkd = nc.dram_tensor("bkd", [128, 2], F32, kind="ExternalInput")
    if use_vb:
        bvd = nc.dram_tensor("bvd", [128, LCOL], F32, kind="ExternalInput")
    if use_ab:
        expb = nc.dram_tensor("expb", [S, S], F32, kind="ExternalInput")
    out = nc.dram_tensor("out", [S, HID], BF16, kind="ExternalOutput")

    with tile.TileContext(nc) as tc:
        with (
            tc.tile_pool(name="per", bufs=1) as per,
            tc.tile_pool(name="ex", bufs=10) as exp_pool,
            tc.tile_pool(name="asc", bufs=4) as asc_pool,
            tc.tile_pool(name="ps", bufs=2, space="PSUM") as ps,
        ):
            # ---- persistent SBUF residents ----
            # xT_sb layout: head region (s 0:512, chunk h at h*512), mid
            # region (s 512:1024, 4096 + h*512), tail (s 1024:2048,
            # 8192 + h*1024) — mirrors the packed dram tensors.
            xT_sb = per.tile([128, 8 * S], BF16)
            wq_sb = per.tile([128, 8 * LCOL], BF16)     # hid-chunk h at cols h*LCOL
            wk_sb = per.tile([128, 8 * LCOL], BF16)
            wv_sb = per.tile([128, 8 * LCOL], BF16)
            wo_sb = per.tile([128, 2 * HID], BF16)      # col-chunk c at cols c*HID
            rotm_sb = per.tile([128, S], BF16)
            qT_sb = per.tile([128, 2 * S], BF16)        # col-chunk c at cols c*S
            kT_sb = per.tile([128, 2 * S], BF16)
            v_sb = per.tile([128, 16 * (HPC * 65)], BF16)  # k-chunk kc: 4 heads x 65
            attnT_sb = per.tile([128, 2 * S], BF16)     # raw (unnormalized) attn^T
            asc_sb = per.tile([128, 2 * S], BF16)       # normalized attn^T
            ones_sb = per.tile([128, 512], BF16)        # warmup operands
            onesf_sb = per.tile([1, 128], F32)          # fp32 broadcast ones

            # warmup first: full-array matmuls so the PE HAM clock gate sees
            # real activity and releases to 2.4 GHz before the projections.
            nc.vector.memset(ones_sb[:], 1.0)
            nc.vector.memset(onesf_sb[:], 1.0)
            wu = ps.tile([128, 512], F32, tag="op", bufs=1, name="wu")
            for i in range(8):
                nc.tensor.matmul(wu[:, 0:448], ones_sb[:, 0:128],
                                 ones_sb[:, 0:448],
                                 start=(i == 0), stop=(i == 7))

            # ---- input DMAs, ordered for earliest first-window start ----
            nc.sync.dma_start(wq_sb[:], wq[:])
            nc.sync.dma_start(xT_sb[:, 0:4096], xh[:])
            nc.sync.dma_start(rotm_sb[:], rotm[:])
            nc.sync.dma_start(wk_sb[:], wk[:])
            nc.sync.dma_start(wv_sb[:], wv[:])
            nc.sync.dma_start(xT_sb[:, 4096:8192], xm[:])
            nc.sync.dma_start(xT_sb[:, 8192:16384], xt2[:])
            nc.sync.dma_start(wo_sb[:], wo[:])
            if use_qkb:
                bq_sb = per.tile([128, 2], F32)
                bk_sb = per.tile([128, 2], F32)
                nc.sync.dma_start(bq_sb[:], bqd[:])
                nc.sync.dma_start(bk_sb[:], bkd[:])
            if use_vb:
                bv_sb = per.tile([128, LCOL], F32)
                nc.sync.dma_start(bv_sb[:], bvd[:])

            # ones columns of v (65th col of each head block)
            v_blocks = v_sb[:].rearrange("p (j c) -> p j c", c=65)
            nc.vector.memset(v_blocks[:, :, 64:65], 1.0)

            # ---- projection pieces ----
            def x_ap(h, s0, width):
                """xT slice for hid-chunk h, s-range [s0, s0+width) — the
                range never crosses a region boundary (512-aligned use)."""
                if s0 < 512:
                    base = h * 512 + s0
                elif s0 < 1024:
                    base = 4096 + h * 512 + (s0 - 512)
                else:
                    base = 8192 + h * 1024 + (s0 - 1024)
                return xT_sb[:, base:base + width]

            def proj_qk(which, c, s4):
                """q/k projection for 512-wide s-block s4 of col-chunk c:
                8 accumulating matmuls, then rotary multiply into qT/kT."""
                w_sb, dst = (wq_sb, qT_sb) if which == "q" else (wk_sb, kT_sb)
                s0 = s4 * 512
                pp = ps.tile([128, 512], F32, tag="pp", bufs=1,
                             name=f"pp_{which}{c}{s4}")
                for h in range(8):
                    nc.tensor.matmul(
                        pp[:],
                        w_sb[:, h * LCOL + c * 128:h * LCOL + (c + 1) * 128],
                        x_ap(h, s0, 512),
                        start=(h == 0),
                        stop=(h == 7),
                    )
                if use_qkb:
                    bias_ap = (bq_sb if which == "q" else bk_sb)[:, c:c + 1]
                    nc.scalar.add(pp[:], pp[:], bias_ap)
                nc.vector.tensor_mul(
                    dst[:, c * S + s0:c * S + s0 + 512],
                    pp[:],
                    rotm_sb[:, s0:s0 + 512],
                )

            def proj_v(j):
                """v projection for s-chunk j (128 rows): 8 N=256 matmuls."""
                vp = ps.tile([128, LCOL], F32, tag="pp", bufs=1,
                             name=f"vp_{j}")
                for h in range(8):
                    nc.tensor.matmul(
                        vp[:],
                        x_ap(h, j * 128, 128),
                        wv_sb[:, h * LCOL:(h + 1) * LCOL],
                        start=(h == 0),
                        stop=(h == 7),
                    )
                dst = v_sb[:, j * (HPC * 65):(j + 1) * (HPC * 65)].rearrange(
                    "p (h c) -> p h c", c=65
                )[:, :, 0:64]
                src = vp[:].rearrange("p (h c) -> p h c", c=64)
                if use_vb:
                    nc.vector.tensor_add(
                        dst, src, bv_sb[:].rearrange("p (h c) -> p h c", c=64)
                    )
                else:
                    nc.vector.tensor_copy(dst, src)

            def out_proj_half(j, n, last_grp):
                """out[j*128:(j+1)*128, n*512:(n+1)*512] = sum_c asc@wo."""
                # alternate PSUM banks so consecutive halves double-buffer
                op = ps.tile([128, 512], F32,
                             tag=("op" if (2 * j + n) % 2 == 0 else "pp"),
                             bufs=1, name=f"op{j}{n}")
                for c in range(2):
                    nc.tensor.matmul(
                        op[:],
                        asc_sb[:, c * S + j * 128:c * S + (j + 1) * 128],
                        wo_sb[:, c * HID + n * 512:c * HID + (n + 1) * 512],
                        start=(c == 0),
                        stop=(c == 1),
                    )
                ost = asc_pool.tile([128, 512], BF16, tag="ost", bufs=4,
                                    name=f"ost{j}{n}")
                if last_grp and (2 * j + n) % 2 == 0:
                    # ScalarE is idle after the final exp; split the tail
                    # evacuations across ScalarE and VectorE
                    nc.scalar.copy(ost[:], op[:])
                else:
                    nc.vector.tensor_copy(ost[:], op[:])
                dma_eng = nc.sync if (j + n) % 2 == 0 else nc.gpsimd
                dma_eng.dma_start(
                    out[j * 128:(j + 1) * 128, n * 512:(n + 1) * 512],
                    ost[:],
                )

            # ---- fixed-step fill schedule (emission order == dep order) ----
            fills = [[] for _ in range(N_STEPS)]

            def at(t, fn):
                fills[max(0, min(t, N_STEPS - 1))].append(fn)

            # v(j) emitted at step j; consumed by the PV drain at step j+LAG.
            for j in range(16):
                at(j, lambda j=j: proj_v(j))
            # k(0,s4) emitted 4 steps before its first consuming scores.
            at(0, lambda: proj_qk("k", 0, 1))
            at(4, lambda: proj_qk("k", 0, 2))
            at(8, lambda: proj_qk("k", 0, 3))
            # q(0,qb) emitted a few steps before window (0,qb).
            at(12, lambda: proj_qk("q", 0, 1))
            at(27, lambda: proj_qk("q", 0, 2))
            at(43, lambda: proj_qk("q", 0, 3))
            # k(1,*) and q(1,0) during windows 2-3.
            at(34, lambda: proj_qk("k", 1, 0))
            at(38, lambda: proj_qk("k", 1, 1))
            at(42, lambda: proj_qk("k", 1, 2))
            at(46, lambda: proj_qk("k", 1, 3))
            at(50, lambda: proj_qk("q", 1, 0))
            at(74, lambda: proj_qk("q", 1, 1))
            at(90, lambda: proj_qk("q", 1, 2))
            at(106, lambda: proj_qk("q", 1, 3))
            # out-proj groups 0-2 into windows 5-7, strictly after group g's
            # c1 normalize (norm_b of window 4+g runs at step (5+g)*16+4).
            for g in range(3):
                base = (5 + g) * 16 + 5
                i = 0
                for j in range(4 * g, 4 * g + 4):
                    for n in range(2):
                        at(base + i, lambda j=j, n=n: out_proj_half(j, n, False))
                        i += 1

            # ---- head: first window's q/k pieces ----
            proj_qk("q", 0, 0)
            proj_qk("k", 0, 0)

            deferred_norm_a = None
            deferred_norm_b = None

            def make_norm(w, p, qb, w0, outT, last_win):
                state = {}

                def norm_a():
                    # 1/den, then broadcast to the two 64-row head substripes
                    # (gpsimd; latency hidden by the norm_b deferral).
                    rt = asc_pool.tile([1, 1024], F32, tag="rt", bufs=2,
                                       name=f"rt{w}")
                    dt = asc_pool.tile([1, 1024], F32, tag="dt", bufs=2,
                                       name=f"dt{w}")
                    if last_win:
                        # ScalarE is idle after the final exp
                        nc.scalar.copy(dt[:, 0:512], outT[0][64:65, :])
                        nc.scalar.copy(dt[:, 512:1024], outT[1][64:65, :])
                    else:
                        nc.vector.tensor_copy(dt[:, 0:512], outT[0][64:65, :])
                        nc.vector.tensor_copy(dt[:, 512:1024],
                                              outT[1][64:65, :])
                    nc.vector.reciprocal_approx_fast(rt[:], dt[:])
                    if last_win:
                        # low-latency path: bf16 K=1 PE matmuls into PSUM
                        rtb = asc_pool.tile([1, 1024], BF16, tag="rtb", bufs=2,
                                            name=f"rtb{w}")
                        nc.vector.tensor_copy(rtb[:], rt[:])
                        rb = ps.tile([128, 512], F32, tag="op", bufs=1,
                                     name=f"rb{w}")
                        nc.tensor.matmul(rb[0:64, :], ones_sb[0:1, 0:64],
                                         rtb[0:1, 0:512],
                                         start=True, stop=True)
                        nc.tensor.matmul(rb[64:128, :], ones_sb[0:1, 64:128],
                                         rtb[0:1, 512:1024],
                                         start=True, stop=True,
                                         skip_group_check=True)
                    else:
                        rb = asc_pool.tile([128, 512], F32, tag="rb", bufs=2,
                                           name=f"rb{w}")
                        rbt = asc_pool.tile([64, 512], F32, tag="rbt", bufs=2,
                                            name=f"rbt{w}")
                        nc.gpsimd.partition_broadcast(rb[0:64, :],
                                                      rt[:, 0:512],
                                                      channels=64)
                        nc.gpsimd.partition_broadcast(rbt[:],
                                                      rt[:, 512:1024],
                                                      channels=64)
                        nc.gpsimd.dma_start(rb[64:128, :], rbt[:])
                    state["rb"] = rb

                def norm_b():
                    nc.vector.tensor_mul(asc_sb[:, w0:w0 + 512],
                                         attnT_sb[:, w0:w0 + 512],
                                         state["rb"][:])
                return norm_a, norm_b

            # ---- main loop: 8 windows x 16 key chunks ----
            for w in range(2 * NW):
                p, qb = divmod(w, NW)
                w0 = p * S + qb * QW
                outT = [
                    ps.tile([65, 512], F32, tag="outT", bufs=2,
                            name=f"outT{w}{hi}")
                    for hi in range(2)
                ]
                pend = []  # (exp_tile, kc) awaiting PV
                for kc in range(16):
                    t = w * 16 + kc
                    sc = ps.tile([128, 1024], F32, tag="sc", name=f"sc{w}{kc}")
                    for hi in range(2):
                        off = hi * 64
                        nc.tensor.matmul(
                            sc[:, hi * 512:(hi + 1) * 512],
                            kT_sb[off:off + 64,
                                  p * S + kc * 128:p * S + (kc + 1) * 128],
                            qT_sb[off:off + 64, w0:w0 + QW],
                            start=True,
                            stop=True,
                        )
                    ex = exp_pool.tile([128, 1024], BF16, tag="ex",
                                       name=f"ex{w}{kc}")
                    nc.scalar.activation(
                        ex[:], sc[:], mybir.ActivationFunctionType.Exp,
                        scale=0.125,
                    )
                    if use_ab:
                        ebt = exp_pool.tile([128, 512], F32, tag="ebt",
                                            bufs=2, name=f"ebt{w}{kc}")
                        nc.sync.dma_start(
                            ebt[:],
                            expb[kc * 128:(kc + 1) * 128,
                                 qb * QW:(qb + 1) * QW],
                        )
                        for hi in range(2):
                            nc.vector.tensor_mul(
                                ex[:, hi * 512:(hi + 1) * 512],
                                ex[:, hi * 512:(hi + 1) * 512],
                                ebt[:],
                            )
                    pend.append((ex, kc))
                    lag = 1 if (w == 2 * NW - 1 and kc == 15) else PV_LAG
                    while len(pend) > lag:
                        exq, kcq = pend.pop(0)
                        for hi in range(2):
                            hq = 2 * p + hi
                            nc.tensor.matmul(
                                outT[hi][:],
                                v_sb[:, kcq * (HPC * 65) + hq * 65:
                                     kcq * (HPC * 65) + hq * 65 + 65],
                                exq[:, hi * 512:(hi + 1) * 512],
                                start=(kcq == 0),
                                stop=(kcq == 15),
                            )
                    if kc == 1 and deferred_norm_a is not None:
                        deferred_norm_a()
                        deferred_norm_a = None
                    if kc == 4 and deferred_norm_b is not None:
                        deferred_norm_b()
                        deferred_norm_b = None
                    for fn in fills[t]:
                        fn()
                for exq, kcq in pend:
                    for hi in range(2):
                        hq = 2 * p + hi
                        nc.tensor.matmul(
                            outT[hi][:],
                            v_sb[:, kcq * (HPC * 65) + hq * 65:
                                 kcq * (HPC * 65) + hq * 65 + 65],
                            exq[:, hi * 512:(hi + 1) * 512],
                            start=(kcq == 0),
                            stop=(kcq == 15),
                        )
                pend = []
                # ---- epilogue phase 1: evacuate outT (frees PSUM) ----
                last_win = w == 2 * NW - 1
                nc.vector.tensor_copy(attnT_sb[0:64, w0:w0 + 512],
                                      outT[0][0:64, :])
                atmp = asc_pool.tile([64, 512], BF16, tag="atmp", bufs=2,
                                     name=f"atmp{w}")
                nc.vector.tensor_copy(atmp[:], outT[1][0:64, :])
                nc.gpsimd.dma_start(attnT_sb[64:128, w0:w0 + 512], atmp[:])
                # phase 2 (den recip/broadcast/normalize) deferred into the
                # next window so it does not head-of-line-block VectorE.
                norm_a, norm_b = make_norm(w, p, qb, w0, outT, last_win)
                if last_win:
                    norm_a()
                    norm_b()
                else:
                    deferred_norm_a = norm_a
                    deferred_norm_b = norm_b

            # ---- tail: last out-proj group after the final normalize ----
            for j in range(12, 16):
                for n in range(2):
                    out_proj_half(j, n, True)

    nc.compile()
    return nc


def _prep_core(c, x, sinusoids, attention_bias, Wq, bq, Wk, bk, Wv, bv, Wo,
               use_qkb, use_vb, use_ab):
    b, g = divmod(c, HPC)
    cols = slice(g * LCOL, (g + 1) * LCOL)
    bf = ml_dtypes.bfloat16
    m = {}
    xr = np.ascontiguousarray(x[b].T).reshape(8, 128, S)
    m["xh"] = np.ascontiguousarray(
        xr[:, :, 0:512].transpose(1, 0, 2).reshape(128, 8 * 512)).astype(bf)
    m["xm"] = np.ascontiguousarray(
        xr[:, :, 512:1024].transpose(1, 0, 2).reshape(128, 8 * 512)).astype(bf)
    m["xt2"] = np.ascontiguousarray(
        xr[:, :, 1024:2048].transpose(1, 0, 2).reshape(128, 8 * 1024)).astype(bf)
    def shuf(w):  # [1024, n] -> [128, 8*n]: row p = concat_h w[h*128+p, :]
        n = w.shape[1]
        return np.ascontiguousarray(
            w.reshape(8, 128, n).transpose(1, 0, 2).reshape(128, 8 * n))
    m["wq"] = shuf(Wq[:, cols]).astype(bf)
    m["wk"] = shuf(Wk[:, cols]).astype(bf)
    m["wv"] = shuf(Wv[:, cols]).astype(bf)
    wo_l = Wo[cols, :]  # [256, 1024] -> [128, 2*1024]
    m["wo"] = np.ascontiguousarray(
        wo_l.reshape(2, 128, HID).transpose(1, 0, 2).reshape(128, 2 * HID)
    ).astype(bf)
    sign = np.where(np.arange(ROT) % 2 == 0, -1.0, 1.0).astype(np.float32)
    mult = sinusoids[b, 1] + sign[None, :] * sinusoids[b, 0]   # [S, ROT]
    rotm = np.ones((128, S), dtype=np.float32)
    rotm[0:ROT] = mult.T
    rotm[64:64 + ROT] = mult.T
    m["rotm"] = rotm.astype(bf)
    if use_qkb:
        m["bqd"] = np.ascontiguousarray(
            bq[cols].reshape(2, 128).T).astype(np.float32)
        m["bkd"] = np.ascontiguousarray(
            bk[cols].reshape(2, 128).T).astype(np.float32)
    if use_vb:
        m["bvd"] = np.broadcast_to(
            bv[cols].astype(np.float32), (128, LCOL)).copy()
    if use_ab:
        m["expb"] = np.ascontiguousarray(
            np.exp(attention_bias[b, 0].astype(np.float32)).T)
    return m


def kernel(x, sinusoids, attention_bias, Wq, bq, Wk, bk, Wv, bv, Wo):
    global LAST_RESULT
    x = np.asarray(x, dtype=np.float32)
    sinusoids = np.asarray(sinusoids, dtype=np.float32)
    attention_bias = np.asarray(attention_bias, dtype=np.float32)
    Wq, Wk, Wv, Wo = (np.asarray(w, dtype=np.float32) for w in (Wq, Wk, Wv, Wo))
    bq, bk, bv = (np.asarray(v, dtype=np.float32) for v in (bq, bk, bv))

    use_qkb = bool(np.any(bq) or np.any(bk))
    use_vb = bool(np.any(bv))
    use_ab = bool(np.any(attention_bias))

    nc = _build(use_qkb, use_vb, use_ab)
    in_maps = [
        _prep_core(c, x, sinusoids, attention_bias, Wq, bq, Wk, bk, Wv, bv, Wo,
                   use_qkb, use_vb, use_ab)
        for c in range(NCORES)
    ]
    import os as _os
    res = run_bass_kernel_spmd(
        nc, in_maps, core_ids=list(range(NCORES)),
        tmpdir=_os.environ.get("BASS_TMPDIR"),
    )
    LAST_RESULT = res
    outs = [r["out"].astype(np.float32) for r in res.results]
    full = np.empty((B, S, HID), dtype=np.float32)
    for b in range(B):
        full[b] = outs[4 * b] + outs[4 * b + 1] + outs[4 * b + 2] + outs[4 * b + 3]
    return full
